# revision 5
# baseline (speedup 1.0000x reference)
"""CausalAttention (B=4, T=2048, C=1024, H=16, D=64) on 8 TRN2 NeuronCores.

Sharding: core c -> (batch b = c//2, head-group hg = c%2 covering heads
hg*8..hg*8+7).  Each core computes QKV for its batch restricted to its 8
heads, causal attention, and the row-sharded output projection partial;
a pairwise ReduceScatter over {2b, 2b+1} sums the two head-group partials
and leaves each core with half the rows of out[b].

Device algorithm (per core, all matmuls fp32r):
  phase A: qkT[j,t] = Wqk^T x^T   (Q,K kept transposed: [channels, T])
           V[t,j]   = x Wv        (stored with a ones-column per head)
  phase B: per q-chunk of 1024, per head:
           sT[k,q] = K_h^T q-block (scores transposed; causal blocks only)
           expT = exp(0.125*sT + causal mask)      (ACT, PSUM->SBUF fp32r)
           out'[d,q] (+ sumexp in row 64) = V'_h^T @ expT   (accumulate over k)
           at[c,q] = out'[0:64]/sumexp  (recip + partition_broadcast + mul)
           proj partial[t,j] = at^T Wproj + bias/2 (per 128-row tile)
           ReduceScatter(add) over the core pair -> rows half -> output
"""
import numpy as np

import concourse.bass as bass
import concourse.tile as tile
from concourse import bacc, mybir
from concourse.bass_utils import run_bass_kernel_spmd

F32 = mybir.dt.float32
F32R = mybir.dt.float32r
AF = mybir.ActivationFunctionType

B, T, C = 4, 2048, 1024
H, D = 16, 64
HL = 8           # heads per core
CL = HL * D      # local channels (512)
NEG = -1e9
CDT = F32R       # matmul compute dtype
QC = 1024        # q-chunk width
NQC = T // QC    # 2
KT = 128         # k-tile
N5 = 512         # matmul free-dim / PSUM bank width (fp32)


def _build():
    nc = bacc.Bacc("TRN2", target_bir_lowering=False, debug=False, num_devices=8)

    xT = nc.dram_tensor("xT", [8, 128, T], CDT, kind="ExternalInput").ap()
    wqk = nc.dram_tensor("wqk", [8, 128, 1024], CDT, kind="ExternalInput").ap()
    wv = nc.dram_tensor("wv", [8, 128, CL], CDT, kind="ExternalInput").ap()
    wproj = nc.dram_tensor("wproj", [4, 128, C], CDT, kind="ExternalInput").ap()
    bias2 = nc.dram_tensor("bias2", [1, C], CDT, kind="ExternalInput").ap()
    ones_r = nc.dram_tensor("ones_r", [1, 128], CDT, kind="ExternalInput").ap()
    ident = nc.dram_tensor("ident", [128, 128], CDT, kind="ExternalInput").ap()
    masks = nc.dram_tensor("masks", [4, 128, N5], CDT, kind="ExternalInput").ap()
    vones = nc.dram_tensor("vones", [128, HL], CDT, kind="ExternalInput").ap()
    out = nc.dram_tensor("out", [T // 2, C], F32, kind="ExternalOutput").ap()

    with tile.TileContext(nc) as tc:
        _emit(nc, tc, xT, wqk, wv, wproj, bias2, ones_r, ident, masks, vones, out)

    nc.compile()
    return nc


def _emit(nc, tc, xT, wqk, wv, wproj, bias2, ones_r, ident, masks, vones, out):
    with tc.tile_pool(name="persist", bufs=1) as pp:
        # qkT[jt]: channels 128*jt..128*jt+127 (j<512: Q; j>=512: K), [128, T]
        qkT = [pp.tile([128, T], CDT, name=f"qkT{j}") for j in range(8)]
        # VV[tb]: [128 t, HL heads, D+1] — col D is the ones column (sumexp trick)
        VV = [pp.tile([128, HL, D + 1], CDT, name=f"VV{t}") for t in range(T // 128)]

        # ---------------- phase A: QKV projections ----------------
        with (
            tc.tile_pool(name="wa", bufs=1) as wa,
            tc.tile_pool(name="xtp", bufs=10) as xtp,
            tc.tile_pool(name="psa", bufs=4, space="PSUM") as psa,
            tc.tile_pool(name="cpa", bufs=4) as cpa,
        ):
            wqk_t = [wa.tile([128, 1024], CDT, name=f"wqk{i}") for i in range(8)]
            wv_t = [wa.tile([128, CL], CDT, name=f"wv{i}") for i in range(8)]
            for i in range(8):
                nc.sync.dma_start(out=wqk_t[i], in_=wqk[i])
                nc.sync.dma_start(out=wv_t[i], in_=wv[i])

            for tch in range(2):  # t-chunks of 1024
                t0 = tch * 1024
                xt = []
                for cb in range(8):
                    x_t = xtp.tile([128, 1024], CDT, tag="xt", name=f"xt{tch}_{cb}")
                    nc.sync.dma_start(out=x_t, in_=xT[cb, :, t0:t0 + 1024])
                    xt.append(x_t)
                # qkT[jt][:, t0:t0+1024]
                for jt in range(8):
                    for s5 in range(2):
                        ps = psa.tile([128, N5], F32, tag="psqk", name=f"pqk{tch}{jt}{s5}")
                        for cb in range(8):
                            nc.tensor.matmul(
                                ps, wqk_t[cb][:, jt * 128:(jt + 1) * 128],
                                xt[cb][:, s5 * N5:(s5 + 1) * N5],
                                start=(cb == 0), stop=(cb == 7))
                        nc.vector.tensor_copy(
                            qkT[jt][:, t0 + s5 * N5: t0 + (s5 + 1) * N5], ps)
                # V tiles for this chunk
                for tb in range(8):
                    gtb = tch * 8 + tb
                    ps = psa.tile([128, CL], F32, tag="psv", name=f"pv{gtb}")
                    for cb in range(8):
                        nc.tensor.matmul(
                            ps, xt[cb][:, tb * 128:(tb + 1) * 128], wv_t[cb],
                            start=(cb == 0), stop=(cb == 7))
                    nc.vector.tensor_copy(
                        VV[gtb][:, :, 0:D],
                        ps.rearrange("p (h d) -> p h d", h=HL))
                    nc.sync.dma_start(
                        out=VV[gtb][:, :, D:D + 1],
                        in_=vones.rearrange("p (h o) -> p h o", o=1))

        # ---------------- phase B: attention + proj + RS ----------------
        with (
            tc.tile_pool(name="wb", bufs=1) as wb,
            tc.tile_pool(name="ps_s", bufs=2, space="PSUM") as ps_s,
            tc.tile_pool(name="ps_av", bufs=4, space="PSUM") as ps_av,
            tc.tile_pool(name="expp", bufs=3) as expp,
            tc.tile_pool(name="atp", bufs=1) as atp,
            tc.tile_pool(name="nrm", bufs=4) as nrm,
            tc.tile_pool(name="stg", bufs=2) as stg,
            tc.tile_pool(name="drp", bufs=2, space="DRAM") as drp,
        ):
            wproj_t = [wb.tile([128, C], CDT, name=f"wproj{i}") for i in range(4)]
            for i in range(4):
                nc.sync.dma_start(out=wproj_t[i], in_=wproj[i])
            bias_t = wb.tile([1, C], CDT, name="bias_t")
            nc.sync.dma_start(out=bias_t, in_=bias2)
            ones_t = wb.tile([1, 128], CDT, name="ones_t")
            nc.sync.dma_start(out=ones_t, in_=ones_r)
            ident_t = wb.tile([128, 128], CDT, name="ident_t")
            nc.sync.dma_start(out=ident_t, in_=ident)
            mask_t = [wb.tile([128, N5], CDT, name=f"mask{i}") for i in range(4)]
            for i in range(4):
                nc.sync.dma_start(out=mask_t[i], in_=masks[i])

            for qc in range(NQC):
                q0 = qc * QC
                nkt = (q0 + QC) // KT  # k-tiles needed: 8 or 16
                at = [atp.tile([128, QC], CDT, tag=f"at{ci}", name=f"at{qc}_{ci}")
                      for ci in range(4)]
                # heads processed in pairs (h at array rows 0-63, h+1 at 64-127)
                for hp in range(HL // 2):
                    heads = (2 * hp, 2 * hp + 1)
                    av = {}
                    for h in heads:
                        for half in range(2):
                            av[(h, half)] = ps_av.tile(
                                [D + 1, N5], F32, tag="av",
                                name=f"av{qc}_{h}_{half}")
                    s_ps = {}
                    for kt in range(nkt):
                        k0 = kt * KT
                        # which 512-halves of this q-chunk are live for this k-tile
                        lives = [half for half in range(2)
                                 if k0 < q0 + half * N5 + N5]
                        off = 0 if 0 in lives else N5
                        wdt = (2 - off // N5) * N5
                        for h in heads:
                            roff = (h % 2) * D
                            jq = h // 2
                            jk = 4 + h // 2
                            sp = ps_s.tile([128, QC], F32, tag="s",
                                           name=f"s{qc}_{hp}_{kt}_{h}")
                            s_ps[h] = sp
                            for half in lives:
                                qh0 = q0 + half * N5
                                dlt = k0 - qh0
                                nc.tensor.matmul(
                                    sp[:, half * N5:(half + 1) * N5],
                                    qkT[jk][roff:roff + D, k0:k0 + KT],
                                    qkT[jq][roff:roff + D, qh0:qh0 + N5],
                                    start=True, stop=(dlt < 0))
                                if dlt >= 0:  # diagonal block: add causal mask
                                    nc.tensor.matmul(
                                        sp[:, half * N5:(half + 1) * N5],
                                        ident_t, mask_t[dlt // KT],
                                        start=False, stop=True)
                        for h in heads:
                            sp = s_ps[h]
                            ex = expp.tile([128, QC], CDT, tag="exp",
                                           name=f"ex{qc}_{hp}_{kt}_{h}")
                            nc.scalar.activation(
                                ex[:, off:off + wdt], sp[:, off:off + wdt],
                                AF.Exp, scale=0.125)
                            for half in lives:
                                last0 = (min(nkt, (q0 + half * N5 + N5) // KT) - 1)
                                nc.tensor.matmul(
                                    av[(h, half)], VV[kt][:, h, :],
                                    ex[:, half * N5:(half + 1) * N5],
                                    start=(kt == 0), stop=(kt == last0))
                    # normalize -> at tiles
                    for h in heads:
                        roff = (h % 2) * D
                        for half in range(2):
                            a = av[(h, half)]
                            # custom-DVE/gpsimd ops need partition-0-aligned
                            # inputs; plain DVE copy handles the 64->0 shift
                            rc0 = nrm.tile([1, N5], F32, tag="rc0",
                                           name=f"rc0{qc}_{h}_{half}")
                            nc.vector.tensor_copy(rc0, a[D:D + 1, :])
                            rc = nrm.tile([1, N5], F32, tag="rc",
                                          name=f"rc{qc}_{h}_{half}")
                            nc.vector.reciprocal_approx_fast(out=rc, in_=rc0)
                            rb = nrm.tile([D, N5], F32, tag="rb",
                                          name=f"rb{qc}_{h}_{half}")
                            nc.gpsimd.partition_broadcast(rb, rc)
                            nc.vector.tensor_mul(
                                at[h // 2][roff:roff + D,
                                           half * N5:(half + 1) * N5],
                                a[0:D, :], rb)
                # ---- projection for this q-chunk
                partial = drp.tile([QC, C], F32, tag="partial", name=f"part{qc}")
                for tt in range(QC // 128):
                    st = stg.tile([128, C], F32, tag="stage", name=f"stg{qc}_{tt}")
                    for jc in range(2):
                        pp_ps = ps_s.tile([128, N5], F32, tag="s",
                                          name=f"pp{qc}_{tt}_{jc}")
                        for ci in range(4):
                            nc.tensor.matmul(
                                pp_ps, at[ci][:, tt * 128:(tt + 1) * 128],
                                wproj_t[ci][:, jc * N5:(jc + 1) * N5],
                                start=(ci == 0), stop=False)
                        nc.tensor.matmul(
                            pp_ps, ones_t, bias_t[0:1, jc * N5:(jc + 1) * N5],
                            start=False, stop=True)
                        nc.vector.tensor_copy(st[:, jc * N5:(jc + 1) * N5], pp_ps)
                    nc.sync.dma_start(
                        out=partial[tt * 128:(tt + 1) * 128, :], in_=st)
                rs_out = drp.tile([QC // 2, C], F32, tag="rsout", name=f"rs{qc}")
                nc.gpsimd.collective_compute(
                    "ReduceScatter", mybir.AluOpType.add,
                    replica_groups=[[0, 1], [2, 3], [4, 5], [6, 7]],
                    ins=[partial[:]], outs=[rs_out[:]])
                nc.sync.dma_start(
                    out=out[qc * (QC // 2):(qc + 1) * (QC // 2), :], in_=rs_out[:])


def _prepare_in_maps(x, Wqkv, Wproj, bproj):
    x = np.asarray(x, dtype=np.float32)
    Wqkv = np.asarray(Wqkv, dtype=np.float32)
    Wproj = np.asarray(Wproj, dtype=np.float32)
    bproj = np.asarray(bproj, dtype=np.float32)

    # causal mask patterns per 128-offset delta within a 512-wide q half
    k_i = np.arange(128)[None, :, None]
    q_i = np.arange(N5)[None, None, :]
    d_i = np.arange(4)[:, None, None]
    masks = np.where(q_i < k_i + 128 * d_i, np.float32(NEG), np.float32(0.0))
    masks = np.ascontiguousarray(masks, dtype=np.float32)

    ident = np.eye(128, dtype=np.float32)
    ones_r = np.ones((1, 128), dtype=np.float32)
    vones = np.ones((128, HL), dtype=np.float32)

    in_maps = []
    for core in range(8):
        b, hg = core // 2, core % 2
        xT = np.ascontiguousarray(x[b].T).reshape(8, 128, T)
        wq = Wqkv[:, hg * CL:(hg + 1) * CL]
        wk = Wqkv[:, C + hg * CL: C + (hg + 1) * CL]
        wv_ = Wqkv[:, 2 * C + hg * CL: 2 * C + (hg + 1) * CL]
        wqk = np.ascontiguousarray(
            np.concatenate([wq, wk], axis=1)).reshape(8, 128, 1024)
        wv = np.ascontiguousarray(wv_).reshape(8, 128, CL)
        wp = np.ascontiguousarray(
            Wproj[hg * CL:(hg + 1) * CL, :]).reshape(4, 128, C)
        in_maps.append({
            "xT": xT, "wqk": wqk, "wv": wv, "wproj": wp,
            "bias2": (bproj / 2.0).reshape(1, C).astype(np.float32),
            "ones_r": ones_r, "ident": ident, "masks": masks, "vones": vones,
        })
    return in_maps


def _assemble(results):
    full = np.empty((B, T, C), dtype=np.float32)
    for core in range(8):
        b, r = core // 2, core % 2
        o = results[core]["out"]  # [1024, 1024]
        for qc in range(NQC):
            full[b, qc * QC + r * (QC // 2): qc * QC + (r + 1) * (QC // 2)] = \
                o[qc * (QC // 2):(qc + 1) * (QC // 2)]
    return full


_NC_CACHE = None


def kernel(x, Wqkv, Wproj, bproj):
    global _NC_CACHE
    if _NC_CACHE is None:
        _NC_CACHE = _build()
    in_maps = _prepare_in_maps(x, Wqkv, Wproj, bproj)
    res = run_bass_kernel_spmd(_NC_CACHE, in_maps, list(range(8)))
    return _assemble(res.results)


# revision 6
# speedup vs baseline: 1.3121x; 1.3121x over previous
"""CausalAttention (B=4, T=2048, C=1024, H=16, D=64) on 8 TRN2 NeuronCores.

Sharding: core c -> (batch b = c//2, head-group hg = c%2 covering heads
hg*8..hg*8+7).  Each core computes QKV for its batch restricted to its 8
heads, causal attention, and the row-sharded output projection partial;
a pairwise ReduceScatter over {2b, 2b+1} sums the two head-group partials
and leaves each core with half the rows of out[b].

Device algorithm (per core, all matmuls fp32r):
  phase A: qkT[j,t] = Wqk^T x^T   (Q,K kept transposed: [channels, T])
           V[t,j]   = x Wv        (stored with a ones-column per head)
  phase B: per q-chunk of 1024, per head:
           sT[k,q] = K_h^T q-block (scores transposed; causal blocks only)
           expT = exp(0.125*sT + causal mask)      (ACT, PSUM->SBUF fp32r)
           out'[d,q] (+ sumexp in row 64) = V'_h^T @ expT   (accumulate over k)
           at[c,q] = out'[0:64]/sumexp  (recip + partition_broadcast + mul)
           proj partial[t,j] = at^T Wproj + bias/2 (per 128-row tile)
           ReduceScatter(add) over the core pair -> rows half -> output
"""
import ml_dtypes
import numpy as np

import concourse.bass as bass
import concourse.tile as tile
from concourse import bacc, mybir
from concourse.bass_utils import run_bass_kernel_spmd

F32 = mybir.dt.float32
F32R = mybir.dt.float32r
AF = mybir.ActivationFunctionType

B, T, C = 4, 2048, 1024
H, D = 16, 64
HL = 8           # heads per core
CL = HL * D      # local channels (512)
NEG = -1e9
CDT = mybir.dt.bfloat16  # matmul compute dtype
QC = 1024        # q-chunk width
NQC = T // QC    # 2
KT = 128         # k-tile
N5 = 512         # matmul free-dim / PSUM bank width (fp32)


def _build():
    nc = bacc.Bacc("TRN2", target_bir_lowering=False, debug=False, num_devices=8)

    xT = nc.dram_tensor("xT", [8, 128, T], CDT, kind="ExternalInput").ap()
    wqk = nc.dram_tensor("wqk", [8, 128, 1024], CDT, kind="ExternalInput").ap()
    wv = nc.dram_tensor("wv", [8, 128, CL], CDT, kind="ExternalInput").ap()
    wproj = nc.dram_tensor("wproj", [4, 128, C], CDT, kind="ExternalInput").ap()
    bias2 = nc.dram_tensor("bias2", [1, C], CDT, kind="ExternalInput").ap()
    ones_r = nc.dram_tensor("ones_r", [1, 128], CDT, kind="ExternalInput").ap()
    ident = nc.dram_tensor("ident", [128, 128], CDT, kind="ExternalInput").ap()
    masks = nc.dram_tensor("masks", [4, 128, N5], CDT, kind="ExternalInput").ap()
    vones = nc.dram_tensor("vones", [128, HL], CDT, kind="ExternalInput").ap()
    out = nc.dram_tensor("out", [T // 2, C], F32, kind="ExternalOutput").ap()

    with tile.TileContext(nc) as tc:
        _emit(nc, tc, xT, wqk, wv, wproj, bias2, ones_r, ident, masks, vones, out)

    nc.compile()
    return nc


def _emit(nc, tc, xT, wqk, wv, wproj, bias2, ones_r, ident, masks, vones, out):
    with tc.tile_pool(name="persist", bufs=1) as pp:
        # qkT[jt]: channels 128*jt..128*jt+127 (j<512: Q; j>=512: K), [128, T]
        qkT = [pp.tile([128, T], CDT, name=f"qkT{j}") for j in range(8)]
        # VV[tb]: [128 t, HL heads, D+1] — col D is the ones column (sumexp trick)
        VV = [pp.tile([128, HL, D + 1], CDT, name=f"VV{t}") for t in range(T // 128)]

        # ---------------- phase A: QKV projections ----------------
        with (
            tc.tile_pool(name="wa", bufs=1) as wa,
            tc.tile_pool(name="xtp", bufs=10) as xtp,
            tc.tile_pool(name="psa", bufs=4, space="PSUM") as psa,
            tc.tile_pool(name="cpa", bufs=4) as cpa,
        ):
            wqk_t = [wa.tile([128, 1024], CDT, name=f"wqk{i}") for i in range(8)]
            wv_t = [wa.tile([128, CL], CDT, name=f"wv{i}") for i in range(8)]
            for i in range(8):
                nc.sync.dma_start(out=wqk_t[i], in_=wqk[i])
                nc.sync.dma_start(out=wv_t[i], in_=wv[i])

            for tch in range(2):  # t-chunks of 1024
                t0 = tch * 1024
                xt = []
                for cb in range(8):
                    x_t = xtp.tile([128, 1024], CDT, tag="xt", name=f"xt{tch}_{cb}")
                    nc.sync.dma_start(out=x_t, in_=xT[cb, :, t0:t0 + 1024])
                    xt.append(x_t)
                # qkT[jt][:, t0:t0+1024]
                for jt in range(8):
                    for s5 in range(2):
                        ps = psa.tile([128, N5], F32, tag="psqk", name=f"pqk{tch}{jt}{s5}")
                        for cb in range(8):
                            nc.tensor.matmul(
                                ps, wqk_t[cb][:, jt * 128:(jt + 1) * 128],
                                xt[cb][:, s5 * N5:(s5 + 1) * N5],
                                start=(cb == 0), stop=(cb == 7))
                        nc.vector.tensor_copy(
                            qkT[jt][:, t0 + s5 * N5: t0 + (s5 + 1) * N5], ps)
                # V tiles for this chunk
                for tb in range(8):
                    gtb = tch * 8 + tb
                    ps = psa.tile([128, CL], F32, tag="psv", name=f"pv{gtb}")
                    for cb in range(8):
                        nc.tensor.matmul(
                            ps, xt[cb][:, tb * 128:(tb + 1) * 128], wv_t[cb],
                            start=(cb == 0), stop=(cb == 7))
                    nc.vector.tensor_copy(
                        VV[gtb][:, :, 0:D],
                        ps.rearrange("p (h d) -> p h d", h=HL))
                    nc.sync.dma_start(
                        out=VV[gtb][:, :, D:D + 1],
                        in_=vones.rearrange("p (h o) -> p h o", o=1))

        # ---------------- phase B: attention + proj + RS ----------------
        with (
            tc.tile_pool(name="wb", bufs=1) as wb,
            tc.tile_pool(name="ps_s", bufs=2, space="PSUM") as ps_s,
            tc.tile_pool(name="ps_av", bufs=4, space="PSUM") as ps_av,
            tc.tile_pool(name="expp", bufs=3) as expp,
            tc.tile_pool(name="atp", bufs=1) as atp,
            tc.tile_pool(name="nrm", bufs=4) as nrm,
            tc.tile_pool(name="stg", bufs=2) as stg,
            tc.tile_pool(name="drp", bufs=2, space="DRAM") as drp,
        ):
            wproj_t = [wb.tile([128, C], CDT, name=f"wproj{i}") for i in range(4)]
            for i in range(4):
                nc.sync.dma_start(out=wproj_t[i], in_=wproj[i])
            bias_t = wb.tile([1, C], CDT, name="bias_t")
            nc.sync.dma_start(out=bias_t, in_=bias2)
            ones_t = wb.tile([1, 128], CDT, name="ones_t")
            nc.sync.dma_start(out=ones_t, in_=ones_r)
            ident_t = wb.tile([128, 128], CDT, name="ident_t")
            nc.sync.dma_start(out=ident_t, in_=ident)
            mask_t = [wb.tile([128, N5], CDT, name=f"mask{i}") for i in range(4)]
            for i in range(4):
                nc.sync.dma_start(out=mask_t[i], in_=masks[i])

            for qc in range(NQC):
                q0 = qc * QC
                nkt = (q0 + QC) // KT  # k-tiles needed: 8 or 16
                at = [atp.tile([128, QC], CDT, tag=f"at{ci}", name=f"at{qc}_{ci}")
                      for ci in range(4)]
                # heads processed in pairs (h at array rows 0-63, h+1 at 64-127)
                for hp in range(HL // 2):
                    heads = (2 * hp, 2 * hp + 1)
                    av = {}
                    for h in heads:
                        for half in range(2):
                            av[(h, half)] = ps_av.tile(
                                [D + 1, N5], F32, tag="av",
                                name=f"av{qc}_{h}_{half}")
                    s_ps = {}
                    for kt in range(nkt):
                        k0 = kt * KT
                        # which 512-halves of this q-chunk are live for this k-tile
                        lives = [half for half in range(2)
                                 if k0 < q0 + half * N5 + N5]
                        off = 0 if 0 in lives else N5
                        wdt = (2 - off // N5) * N5
                        for h in heads:
                            roff = (h % 2) * D
                            jq = h // 2
                            jk = 4 + h // 2
                            sp = ps_s.tile([128, QC], F32, tag="s",
                                           name=f"s{qc}_{hp}_{kt}_{h}")
                            s_ps[h] = sp
                            for half in lives:
                                qh0 = q0 + half * N5
                                dlt = k0 - qh0
                                nc.tensor.matmul(
                                    sp[:, half * N5:(half + 1) * N5],
                                    qkT[jk][roff:roff + D, k0:k0 + KT],
                                    qkT[jq][roff:roff + D, qh0:qh0 + N5],
                                    start=True, stop=(dlt < 0))
                                if dlt >= 0:  # diagonal block: add causal mask
                                    nc.tensor.matmul(
                                        sp[:, half * N5:(half + 1) * N5],
                                        ident_t, mask_t[dlt // KT],
                                        start=False, stop=True)
                        for h in heads:
                            sp = s_ps[h]
                            ex = expp.tile([128, QC], CDT, tag="exp",
                                           name=f"ex{qc}_{hp}_{kt}_{h}")
                            nc.scalar.activation(
                                ex[:, off:off + wdt], sp[:, off:off + wdt],
                                AF.Exp, scale=0.125)
                            for half in lives:
                                last0 = (min(nkt, (q0 + half * N5 + N5) // KT) - 1)
                                nc.tensor.matmul(
                                    av[(h, half)], VV[kt][:, h, :],
                                    ex[:, half * N5:(half + 1) * N5],
                                    start=(kt == 0), stop=(kt == last0))
                    # normalize -> at tiles
                    for h in heads:
                        roff = (h % 2) * D
                        for half in range(2):
                            a = av[(h, half)]
                            # custom-DVE/gpsimd ops need partition-0-aligned
                            # inputs; plain DVE copy handles the 64->0 shift
                            rc0 = nrm.tile([1, N5], F32, tag="rc0",
                                           name=f"rc0{qc}_{h}_{half}")
                            nc.vector.tensor_copy(rc0, a[D:D + 1, :])
                            rc = nrm.tile([1, N5], F32, tag="rc",
                                          name=f"rc{qc}_{h}_{half}")
                            nc.vector.reciprocal_approx_fast(out=rc, in_=rc0)
                            rb = nrm.tile([D, N5], F32, tag="rb",
                                          name=f"rb{qc}_{h}_{half}")
                            nc.gpsimd.partition_broadcast(rb, rc)
                            nc.vector.tensor_mul(
                                at[h // 2][roff:roff + D,
                                           half * N5:(half + 1) * N5],
                                a[0:D, :], rb)
                # ---- projection for this q-chunk
                partial = drp.tile([QC, C], F32, tag="partial", name=f"part{qc}")
                for tt in range(QC // 128):
                    st = stg.tile([128, C], F32, tag="stage", name=f"stg{qc}_{tt}")
                    for jc in range(2):
                        pp_ps = ps_s.tile([128, N5], F32, tag="s",
                                          name=f"pp{qc}_{tt}_{jc}")
                        for ci in range(4):
                            nc.tensor.matmul(
                                pp_ps, at[ci][:, tt * 128:(tt + 1) * 128],
                                wproj_t[ci][:, jc * N5:(jc + 1) * N5],
                                start=(ci == 0), stop=False)
                        nc.tensor.matmul(
                            pp_ps, ones_t, bias_t[0:1, jc * N5:(jc + 1) * N5],
                            start=False, stop=True)
                        nc.vector.tensor_copy(st[:, jc * N5:(jc + 1) * N5], pp_ps)
                    nc.sync.dma_start(
                        out=partial[tt * 128:(tt + 1) * 128, :], in_=st)
                rs_out = drp.tile([QC // 2, C], F32, tag="rsout", name=f"rs{qc}")
                nc.gpsimd.collective_compute(
                    "ReduceScatter", mybir.AluOpType.add,
                    replica_groups=[[0, 1], [2, 3], [4, 5], [6, 7]],
                    ins=[partial[:]], outs=[rs_out[:]])
                nc.sync.dma_start(
                    out=out[qc * (QC // 2):(qc + 1) * (QC // 2), :], in_=rs_out[:])


def _prepare_in_maps(x, Wqkv, Wproj, bproj):
    x = np.asarray(x, dtype=np.float32)
    Wqkv = np.asarray(Wqkv, dtype=np.float32)
    Wproj = np.asarray(Wproj, dtype=np.float32)
    bproj = np.asarray(bproj, dtype=np.float32)

    # causal mask patterns per 128-offset delta within a 512-wide q half
    k_i = np.arange(128)[None, :, None]
    q_i = np.arange(N5)[None, None, :]
    d_i = np.arange(4)[:, None, None]
    masks = np.where(q_i < k_i + 128 * d_i, np.float32(NEG), np.float32(0.0))
    masks = np.ascontiguousarray(masks, dtype=np.float32)

    ident = np.eye(128, dtype=np.float32)
    ones_r = np.ones((1, 128), dtype=np.float32)
    vones = np.ones((128, HL), dtype=np.float32)

    in_maps = []
    for core in range(8):
        b, hg = core // 2, core % 2
        xT = np.ascontiguousarray(x[b].T).reshape(8, 128, T)
        wq = Wqkv[:, hg * CL:(hg + 1) * CL]
        wk = Wqkv[:, C + hg * CL: C + (hg + 1) * CL]
        wv_ = Wqkv[:, 2 * C + hg * CL: 2 * C + (hg + 1) * CL]
        wqk = np.ascontiguousarray(
            np.concatenate([wq, wk], axis=1)).reshape(8, 128, 1024)
        wv = np.ascontiguousarray(wv_).reshape(8, 128, CL)
        wp = np.ascontiguousarray(
            Wproj[hg * CL:(hg + 1) * CL, :]).reshape(4, 128, C)
        bf = ml_dtypes.bfloat16
        in_maps.append({
            "xT": xT.astype(bf), "wqk": wqk.astype(bf), "wv": wv.astype(bf),
            "wproj": wp.astype(bf),
            "bias2": (bproj / 2.0).reshape(1, C).astype(bf),
            "ones_r": ones_r.astype(bf), "ident": ident.astype(bf),
            "masks": masks.astype(bf), "vones": vones.astype(bf),
        })
    return in_maps


def _assemble(results):
    full = np.empty((B, T, C), dtype=np.float32)
    for core in range(8):
        b, r = core // 2, core % 2
        o = results[core]["out"]  # [1024, 1024]
        for qc in range(NQC):
            full[b, qc * QC + r * (QC // 2): qc * QC + (r + 1) * (QC // 2)] = \
                o[qc * (QC // 2):(qc + 1) * (QC // 2)]
    return full


_NC_CACHE = None


def kernel(x, Wqkv, Wproj, bproj):
    global _NC_CACHE
    if _NC_CACHE is None:
        _NC_CACHE = _build()
    in_maps = _prepare_in_maps(x, Wqkv, Wproj, bproj)
    res = run_bass_kernel_spmd(_NC_CACHE, in_maps, list(range(8)))
    return _assemble(res.results)


# revision 7
# speedup vs baseline: 1.4025x; 1.0689x over previous
"""CausalAttention (B=4, T=2048, C=1024, H=16, D=64) on 8 TRN2 NeuronCores.

Sharding: core c -> (batch b = c//2, head-group hg = c%2 covering heads
hg*8..hg*8+7).  Each core computes QKV for its batch restricted to its 8
heads, causal attention, and the row-sharded output projection partial;
a pairwise ReduceScatter over {2b, 2b+1} sums the two head-group partials
and leaves each core with half the rows of out[b].

Device algorithm (per core, all matmuls fp32r):
  phase A: qkT[j,t] = Wqk^T x^T   (Q,K kept transposed: [channels, T])
           V[t,j]   = x Wv        (stored with a ones-column per head)
  phase B: per q-chunk of 1024, per head:
           sT[k,q] = K_h^T q-block (scores transposed; causal blocks only)
           expT = exp(0.125*sT + causal mask)      (ACT, PSUM->SBUF fp32r)
           out'[d,q] (+ sumexp in row 64) = V'_h^T @ expT   (accumulate over k)
           at[c,q] = out'[0:64]/sumexp  (recip + partition_broadcast + mul)
           proj partial[t,j] = at^T Wproj + bias/2 (per 128-row tile)
           ReduceScatter(add) over the core pair -> rows half -> output
"""
import ml_dtypes
import numpy as np

import concourse.bass as bass
import concourse.tile as tile
from concourse import bacc, mybir
from concourse.bass_utils import run_bass_kernel_spmd

F32 = mybir.dt.float32
F32R = mybir.dt.float32r
AF = mybir.ActivationFunctionType

B, T, C = 4, 2048, 1024
H, D = 16, 64
HL = 8           # heads per core
CL = HL * D      # local channels (512)
NEG = -1e9
CDT = mybir.dt.bfloat16  # matmul compute dtype
QC = 1024        # q-chunk width
NQC = T // QC    # 2
KT = 128         # k-tile
N5 = 512         # matmul free-dim / PSUM bank width (fp32)


def _build():
    nc = bacc.Bacc("TRN2", target_bir_lowering=False, debug=False, num_devices=8)

    xT = nc.dram_tensor("xT", [8, 128, T], CDT, kind="ExternalInput").ap()
    wqk = nc.dram_tensor("wqk", [8, 128, 1024], CDT, kind="ExternalInput").ap()
    wv = nc.dram_tensor("wv", [8, 128, CL], CDT, kind="ExternalInput").ap()
    wproj = nc.dram_tensor("wproj", [4, 128, C], CDT, kind="ExternalInput").ap()
    bias2 = nc.dram_tensor("bias2", [1, C], CDT, kind="ExternalInput").ap()
    ones_r = nc.dram_tensor("ones_r", [1, 128], CDT, kind="ExternalInput").ap()
    ident = nc.dram_tensor("ident", [128, 128], CDT, kind="ExternalInput").ap()
    masks = nc.dram_tensor("masks", [4, 128, N5], CDT, kind="ExternalInput").ap()
    vones = nc.dram_tensor("vones", [128, HL], CDT, kind="ExternalInput").ap()
    out = nc.dram_tensor("out", [T // 2, C], F32, kind="ExternalOutput").ap()

    with tile.TileContext(nc) as tc:
        _emit(nc, tc, xT, wqk, wv, wproj, bias2, ones_r, ident, masks, vones, out)

    nc.compile()
    return nc


def _emit(nc, tc, xT, wqk, wv, wproj, bias2, ones_r, ident, masks, vones, out):
    with tc.tile_pool(name="persist", bufs=1) as pp:
        # qkT[jt]: channels 128*jt..128*jt+127 (j<512: Q; j>=512: K), [128, T]
        qkT = [pp.tile([128, T], CDT, name=f"qkT{j}") for j in range(8)]
        # VV[tb]: [128 t, HL heads, D+1] — col D is the ones column (sumexp trick)
        VV = [pp.tile([128, HL, D + 1], CDT, name=f"VV{t}") for t in range(T // 128)]
        # all weights/constants loaded once up-front
        wqk_t = [pp.tile([128, 1024], CDT, name=f"wqk{i}") for i in range(8)]
        wv_t = [pp.tile([128, CL], CDT, name=f"wv{i}") for i in range(8)]
        wproj_t = [pp.tile([128, C], CDT, name=f"wproj{i}") for i in range(4)]
        bias_t = pp.tile([1, C], CDT, name="bias_t")
        ones_t = pp.tile([1, 128], CDT, name="ones_t")
        ident_t = pp.tile([128, 128], CDT, name="ident_t")
        mask_t = [pp.tile([128, N5], CDT, name=f"mask{i}") for i in range(4)]
        for i in range(8):
            nc.sync.dma_start(out=wqk_t[i], in_=wqk[i])
            nc.sync.dma_start(out=wv_t[i], in_=wv[i])
        for i in range(4):
            nc.sync.dma_start(out=wproj_t[i], in_=wproj[i])
            nc.sync.dma_start(out=mask_t[i], in_=masks[i])
        nc.sync.dma_start(out=bias_t, in_=bias2)
        nc.sync.dma_start(out=ones_t, in_=ones_r)
        nc.sync.dma_start(out=ident_t, in_=ident)

        # ---------------- phase A: QKV projections ----------------
        with (
            tc.tile_pool(name="xtp", bufs=16) as xtp,
            tc.tile_pool(name="psa", bufs=4, space="PSUM") as psa,
        ):

            for tch in range(2):  # t-chunks of 1024
                t0 = tch * 1024
                xt = []
                for cb in range(8):
                    x_t = xtp.tile([128, 1024], CDT, tag="xt", name=f"xt{tch}_{cb}")
                    nc.sync.dma_start(out=x_t, in_=xT[cb, :, t0:t0 + 1024])
                    xt.append(x_t)
                # qkT[jt][:, t0:t0+1024]
                for jt in range(8):
                    for s5 in range(2):
                        ps = psa.tile([128, N5], F32, tag="psqk", name=f"pqk{tch}{jt}{s5}")
                        for cb in range(8):
                            nc.tensor.matmul(
                                ps, wqk_t[cb][:, jt * 128:(jt + 1) * 128],
                                xt[cb][:, s5 * N5:(s5 + 1) * N5],
                                start=(cb == 0), stop=(cb == 7))
                        nc.vector.tensor_copy(
                            qkT[jt][:, t0 + s5 * N5: t0 + (s5 + 1) * N5], ps)
                # V tiles for this chunk
                for tb in range(8):
                    gtb = tch * 8 + tb
                    ps = psa.tile([128, CL], F32, tag="psv", name=f"pv{gtb}")
                    for cb in range(8):
                        nc.tensor.matmul(
                            ps, xt[cb][:, tb * 128:(tb + 1) * 128], wv_t[cb],
                            start=(cb == 0), stop=(cb == 7))
                    nc.vector.tensor_copy(
                        VV[gtb][:, :, 0:D],
                        ps.rearrange("p (h d) -> p h d", h=HL))
                    nc.sync.dma_start(
                        out=VV[gtb][:, :, D:D + 1],
                        in_=vones.rearrange("p (h o) -> p h o", o=1))

        # ---------------- phase B: attention + proj + RS ----------------
        with (
            tc.tile_pool(name="ps_s", bufs=2, space="PSUM") as ps_s,
            tc.tile_pool(name="ps_av", bufs=4, space="PSUM") as ps_av,
            tc.tile_pool(name="expp", bufs=6) as expp,
            tc.tile_pool(name="atp", bufs=2) as atp,
            tc.tile_pool(name="avs_p", bufs=4) as avs_p,
            tc.tile_pool(name="nrm", bufs=4) as nrm,
            tc.tile_pool(name="stg", bufs=2) as stg,
            tc.tile_pool(name="drp", bufs=2, space="DRAM") as drp,
        ):
            for qc in range(NQC):
                q0 = qc * QC
                nkt = (q0 + QC) // KT  # k-tiles needed: 8 or 16
                at = [atp.tile([128, QC], CDT, tag=f"at{ci}", name=f"at{qc}_{ci}")
                      for ci in range(4)]
                # heads processed in pairs (h at array rows 0-63, h+1 at 64-127)
                for hp in range(HL // 2):
                    heads = (2 * hp, 2 * hp + 1)
                    av = {}
                    for h in heads:
                        for half in range(2):
                            av[(h, half)] = ps_av.tile(
                                [D + 1, N5], F32, tag="av",
                                name=f"av{qc}_{h}_{half}")
                    exps = {}

                    def emit_scores(kt):
                        k0 = kt * KT
                        lives = [half for half in range(2)
                                 if k0 < q0 + half * N5 + N5]
                        off = 0 if 0 in lives else N5
                        wdt = (2 - off // N5) * N5
                        for h in heads:
                            roff = (h % 2) * D
                            jq = h // 2
                            jk = 4 + h // 2
                            sp = ps_s.tile([128, QC], F32, tag="s",
                                           name=f"s{qc}_{hp}_{kt}_{h}")
                            for half in lives:
                                qh0 = q0 + half * N5
                                dlt = k0 - qh0
                                nc.tensor.matmul(
                                    sp[:, half * N5:(half + 1) * N5],
                                    qkT[jk][roff:roff + D, k0:k0 + KT],
                                    qkT[jq][roff:roff + D, qh0:qh0 + N5],
                                    start=True, stop=(dlt < 0))
                                if dlt >= 0:  # diagonal block: add causal mask
                                    nc.tensor.matmul(
                                        sp[:, half * N5:(half + 1) * N5],
                                        ident_t, mask_t[dlt // KT],
                                        start=False, stop=True)
                            ex = expp.tile([128, QC], CDT, tag="exp",
                                           name=f"ex{qc}_{hp}_{kt}_{h}")
                            nc.scalar.activation(
                                ex[:, off:off + wdt], sp[:, off:off + wdt],
                                AF.Exp, scale=0.125)
                            exps[(h, kt)] = ex

                    def emit_attnv(kt):
                        k0 = kt * KT
                        lives = [half for half in range(2)
                                 if k0 < q0 + half * N5 + N5]
                        for h in heads:
                            ex = exps.pop((h, kt))
                            for half in lives:
                                last0 = (min(nkt, (q0 + half * N5 + N5) // KT) - 1)
                                nc.tensor.matmul(
                                    av[(h, half)], VV[kt][:, h, :],
                                    ex[:, half * N5:(half + 1) * N5],
                                    start=(kt == 0), stop=(kt == last0))

                    # software pipeline: scores run one k-tile ahead of attnV
                    emit_scores(0)
                    for kt in range(1, nkt):
                        emit_scores(kt)
                        emit_attnv(kt - 1)
                    emit_attnv(nkt - 1)

                    # evacuate attnV PSUM quickly, then normalize -> at tiles
                    for h in heads:
                        roff = (h % 2) * D
                        for half in range(2):
                            a = avs_p.tile([D + 1, N5], F32, tag="avs",
                                           name=f"avs{qc}_{h}_{half}")
                            nc.vector.tensor_copy(a, av[(h, half)])
                            # custom-DVE/gpsimd ops need partition-0-aligned
                            # inputs; plain DVE copy handles the 64->0 shift
                            rc0 = nrm.tile([1, N5], F32, tag="rc0",
                                           name=f"rc0{qc}_{h}_{half}")
                            nc.vector.tensor_copy(rc0, a[D:D + 1, :])
                            rc = nrm.tile([1, N5], F32, tag="rc",
                                          name=f"rc{qc}_{h}_{half}")
                            nc.vector.reciprocal_approx_fast(out=rc, in_=rc0)
                            rb = nrm.tile([D, N5], F32, tag="rb",
                                          name=f"rb{qc}_{h}_{half}")
                            nc.gpsimd.partition_broadcast(rb, rc)
                            nc.vector.tensor_mul(
                                at[h // 2][roff:roff + D,
                                           half * N5:(half + 1) * N5],
                                a[0:D, :], rb)
                # ---- projection for this q-chunk
                partial = drp.tile([QC, C], F32, tag="partial", name=f"part{qc}")
                for tt in range(QC // 128):
                    st = stg.tile([128, C], F32, tag="stage", name=f"stg{qc}_{tt}")
                    for jc in range(2):
                        pp_ps = ps_s.tile([128, N5], F32, tag="s",
                                          name=f"pp{qc}_{tt}_{jc}")
                        for ci in range(4):
                            nc.tensor.matmul(
                                pp_ps, at[ci][:, tt * 128:(tt + 1) * 128],
                                wproj_t[ci][:, jc * N5:(jc + 1) * N5],
                                start=(ci == 0), stop=False)
                        nc.tensor.matmul(
                            pp_ps, ones_t, bias_t[0:1, jc * N5:(jc + 1) * N5],
                            start=False, stop=True)
                        nc.vector.tensor_copy(st[:, jc * N5:(jc + 1) * N5], pp_ps)
                    nc.sync.dma_start(
                        out=partial[tt * 128:(tt + 1) * 128, :], in_=st)
                rs_out = drp.tile([QC // 2, C], F32, tag="rsout", name=f"rs{qc}")
                nc.gpsimd.collective_compute(
                    "ReduceScatter", mybir.AluOpType.add,
                    replica_groups=[[0, 1], [2, 3], [4, 5], [6, 7]],
                    ins=[partial[:]], outs=[rs_out[:]])
                nc.sync.dma_start(
                    out=out[qc * (QC // 2):(qc + 1) * (QC // 2), :], in_=rs_out[:])


def _prepare_in_maps(x, Wqkv, Wproj, bproj):
    x = np.asarray(x, dtype=np.float32)
    Wqkv = np.asarray(Wqkv, dtype=np.float32)
    Wproj = np.asarray(Wproj, dtype=np.float32)
    bproj = np.asarray(bproj, dtype=np.float32)

    # causal mask patterns per 128-offset delta within a 512-wide q half
    k_i = np.arange(128)[None, :, None]
    q_i = np.arange(N5)[None, None, :]
    d_i = np.arange(4)[:, None, None]
    masks = np.where(q_i < k_i + 128 * d_i, np.float32(NEG), np.float32(0.0))
    masks = np.ascontiguousarray(masks, dtype=np.float32)

    ident = np.eye(128, dtype=np.float32)
    ones_r = np.ones((1, 128), dtype=np.float32)
    vones = np.ones((128, HL), dtype=np.float32)

    in_maps = []
    for core in range(8):
        b, hg = core // 2, core % 2
        xT = np.ascontiguousarray(x[b].T).reshape(8, 128, T)
        wq = Wqkv[:, hg * CL:(hg + 1) * CL]
        wk = Wqkv[:, C + hg * CL: C + (hg + 1) * CL]
        wv_ = Wqkv[:, 2 * C + hg * CL: 2 * C + (hg + 1) * CL]
        wqk = np.ascontiguousarray(
            np.concatenate([wq, wk], axis=1)).reshape(8, 128, 1024)
        wv = np.ascontiguousarray(wv_).reshape(8, 128, CL)
        wp = np.ascontiguousarray(
            Wproj[hg * CL:(hg + 1) * CL, :]).reshape(4, 128, C)
        bf = ml_dtypes.bfloat16
        in_maps.append({
            "xT": xT.astype(bf), "wqk": wqk.astype(bf), "wv": wv.astype(bf),
            "wproj": wp.astype(bf),
            "bias2": (bproj / 2.0).reshape(1, C).astype(bf),
            "ones_r": ones_r.astype(bf), "ident": ident.astype(bf),
            "masks": masks.astype(bf), "vones": vones.astype(bf),
        })
    return in_maps


def _assemble(results):
    full = np.empty((B, T, C), dtype=np.float32)
    for core in range(8):
        b, r = core // 2, core % 2
        o = results[core]["out"]  # [1024, 1024]
        for qc in range(NQC):
            full[b, qc * QC + r * (QC // 2): qc * QC + (r + 1) * (QC // 2)] = \
                o[qc * (QC // 2):(qc + 1) * (QC // 2)]
    return full


_NC_CACHE = None


def kernel(x, Wqkv, Wproj, bproj):
    global _NC_CACHE
    if _NC_CACHE is None:
        _NC_CACHE = _build()
    in_maps = _prepare_in_maps(x, Wqkv, Wproj, bproj)
    res = run_bass_kernel_spmd(_NC_CACHE, in_maps, list(range(8)))
    return _assemble(res.results)
